# revision 6
# baseline (speedup 1.0000x reference)
"""Top-1 MoE (8 experts) expert-parallel kernel for Trainium2, 8 NeuronCores.

Strategy:
  - Host: argmax(router_logits) -> per-token expert id; gather each expert's
    tokens (the "all-to-all dispatch" happens host-side since we receive full
    inputs and return full outputs).
  - Device (SPMD, one expert per core): dense 2-GEMM SiLU MLP in bf16 with
    fp32 PSUM accumulation. Weights are streamed through SBUF; activations
    (x, h) are SBUF-resident.
  - Host: scatter each expert's outputs back to token order ("combine").

Per-core problem: x[C, D] @ w1[F, D].T -> silu -> @ w2[D, F].T, with
D=2048, F=4096, C = padded max token count per expert (multiple of 128).

Device layouts (partition-major so every DMA is a plain slice):
  xt  [128, 16, C]  bf16   xt[p, ko, t]  = x[t, ko*128+p]        (lhs-T of x)
  w1t [128, 16, F]  bf16   w1t[p, ko, f] = w1[f, ko*128+p]       (k-major w1)
  w2t [128, 32, D]  bf16   w2t[p, ko, d] = w2[d, ko*128+p]       (k-major w2)
  yt  [128, 16, C]  f32    yt[p, do, t]  = y[t, do*128+p]
"""

import numpy as np
import ml_dtypes

BF16 = ml_dtypes.bfloat16

P = 128
D = 2048
F = 4096
E = 8
N_CORES = 8
TCHUNK = 512  # token chunk = matmul free dim (one PSUM bank of fp32)
W1B = 512     # GEMM1 weight block width (columns of F per streamed tile)
W2B = 256     # GEMM2 weight block width (columns of D per streamed tile)

KO1 = D // P  # 16 contraction tiles for GEMM1
KO2 = F // P  # 32 contraction tiles for GEMM2

_BUILD_CACHE = {}


def _token_chunks(C):
    out = []
    t0 = 0
    while t0 < C:
        tw = min(TCHUNK, C - t0)
        out.append((t0, tw))
        t0 += tw
    return out


def build_nc(C, act="silu"):
    """Build + compile the per-core Bass program for token capacity C."""
    key = (C, act)
    if key in _BUILD_CACHE:
        return _BUILD_CACHE[key]

    import concourse.bacc as bacc
    import concourse.mybir as mybir
    from concourse import tile

    dt = mybir.dt
    act_fn = {
        "silu": mybir.ActivationFunctionType.Silu,
        "sigmoid": mybir.ActivationFunctionType.Sigmoid,
    }[act]
    nc = bacc.Bacc("TRN2", target_bir_lowering=False, debug=False)

    xt_d = nc.dram_tensor("xt", [P, KO1, C], dt.bfloat16, kind="ExternalInput")
    w1t_d = nc.dram_tensor("w1t", [P, KO1, F], dt.bfloat16, kind="ExternalInput")
    w2t_d = nc.dram_tensor("w2t", [P, KO2, D], dt.bfloat16, kind="ExternalInput")
    yt_d = nc.dram_tensor("yt", [P, KO1, C], dt.float32, kind="ExternalOutput")

    chunks = _token_chunks(C)
    N1 = F // W1B
    N2 = D // W2B

    with tile.TileContext(nc) as tc:
        with (
            tc.tile_pool(name="xpool", bufs=1) as xpool,
            tc.tile_pool(name="hpool", bufs=1) as hpool,
            tc.tile_pool(name="wpool", bufs=2) as wpool,
            tc.tile_pool(name="ypool", bufs=4) as ypool,
            tc.tile_pool(name="cpool", bufs=1) as cpool,
            tc.tile_pool(name="pspool", bufs=4, space="PSUM") as pspool,
        ):
            zbias = cpool.tile([P, 1], dt.float32)
            nc.any.memset(zbias[:], 0.0)

            x_sb = xpool.tile([P, KO1, C], dt.bfloat16)
            h_sb = hpool.tile([P, KO2, C], dt.bfloat16)

            # Load x by token-chunk so GEMM1 can start after the first chunk.
            for (t0, tw) in chunks:
                nc.sync.dma_start(x_sb[:, :, t0 : t0 + tw], xt_d[:, :, t0 : t0 + tw])

            # GEMM1 + SiLU: h[f, t] = silu(sum_d w1t[d, f] * x[d, t])
            for mb in range(N1):
                w1_sb = wpool.tile(
                    [P, KO1, W1B], dt.bfloat16, tag="w", name=f"w1_{mb}"
                )
                nc.sync.dma_start(w1_sb[:], w1t_d[:, :, mb * W1B : (mb + 1) * W1B])
                for (t0, tw) in chunks:
                    for ms in range(W1B // P):
                        ps = pspool.tile(
                            [P, TCHUNK], dt.float32, tag="ps", name=f"ps1_{mb}_{t0}_{ms}"
                        )
                        for k in range(KO1):
                            nc.tensor.matmul(
                                ps[:, :tw],
                                w1_sb[:, k, ms * P : (ms + 1) * P],
                                x_sb[:, k, t0 : t0 + tw],
                                start=(k == 0),
                                stop=(k == KO1 - 1),
                            )
                        fo = mb * (W1B // P) + ms
                        nc.scalar.activation(
                            h_sb[:, fo, t0 : t0 + tw],
                            ps[:, :tw],
                            act_fn,
                            bias=zbias[:],
                        )

            # GEMM2: y[d, t] = sum_f w2t[f, d] * h[f, t]
            for db in range(N2):
                w2_sb = wpool.tile(
                    [P, KO2, W2B], dt.bfloat16, tag="w", name=f"w2_{db}"
                )
                nc.sync.dma_start(w2_sb[:], w2t_d[:, :, db * W2B : (db + 1) * W2B])
                for (t0, tw) in chunks:
                    for ds in range(W2B // P):
                        ps = pspool.tile(
                            [P, TCHUNK], dt.float32, tag="ps", name=f"ps2_{db}_{t0}_{ds}"
                        )
                        for k in range(KO2):
                            nc.tensor.matmul(
                                ps[:, :tw],
                                w2_sb[:, k, ds * P : (ds + 1) * P],
                                h_sb[:, k, t0 : t0 + tw],
                                start=(k == 0),
                                stop=(k == KO2 - 1),
                            )
                        do = db * (W2B // P) + ds
                        y_sb = ypool.tile(
                            [P, TCHUNK], dt.float32, tag="y", name=f"y_{db}_{t0}_{ds}"
                        )
                        nc.vector.tensor_copy(y_sb[:, :tw], ps[:, :tw])
                        nc.sync.dma_start(yt_d[:, do, t0 : t0 + tw], y_sb[:, :tw])

    nc.compile()
    _BUILD_CACHE[key] = nc
    return nc


def _pack_tokens(x_e, C):
    """x_e [n, D] f32 -> xt [128, KO1, C] bf16 (zero padded)."""
    n = x_e.shape[0]
    xb = np.zeros((C, D), dtype=BF16)
    xb[:n] = x_e.astype(BF16)
    return np.ascontiguousarray(xb.reshape(C, KO1, P).transpose(2, 1, 0))


def _pack_w1(w1_e):
    """w1_e [F, D] f32 -> [128, KO1, F] bf16."""
    return np.ascontiguousarray(
        w1_e.astype(BF16).reshape(F, KO1, P).transpose(2, 1, 0)
    )


def _pack_w2(w2_e):
    """w2_e [D, F] f32 -> [128, KO2, D] bf16."""
    return np.ascontiguousarray(
        w2_e.astype(BF16).reshape(D, KO2, P).transpose(2, 1, 0)
    )


LAST_RUN = {}


def prepare(hidden_states, router_logits, w1, w2):
    """Host-side routing + packing. Returns (nc, in_maps, meta)."""
    hidden_states = np.asarray(hidden_states)
    router_logits = np.asarray(router_logits)
    w1 = np.asarray(w1)
    w2 = np.asarray(w2)

    b, s, d = hidden_states.shape
    T = b * s
    x = hidden_states.reshape(T, d).astype(np.float32)
    assign = np.argmax(router_logits.reshape(T, E), axis=-1)

    idx = [np.nonzero(assign == e)[0] for e in range(E)]
    counts = np.array([i.size for i in idx])
    C = max(P, int(-(-counts.max() // P)) * P)  # pad capacity to multiple of 128

    nc = build_nc(C)

    in_maps = []
    for e in range(E):
        in_maps.append(
            {
                "xt": _pack_tokens(x[idx[e]], C),
                "w1t": _pack_w1(w1[e]),
                "w2t": _pack_w2(w2[e]),
            }
        )
    meta = {"b": b, "s": s, "d": d, "T": T, "C": C, "idx": idx, "counts": counts}
    return nc, in_maps, meta


def finish(results, meta):
    """Scatter per-expert outputs back to token order."""
    T, d, C = meta["T"], meta["d"], meta["C"]
    out = np.zeros((T, d), dtype=np.float32)
    for e in range(E):
        yt = np.asarray(results[e]["yt"])  # [128, KO1, C] f32
        y_tok = yt.transpose(2, 1, 0).reshape(C, D)
        out[meta["idx"][e]] = y_tok[: meta["counts"][e]]
    return out.reshape(meta["b"], meta["s"], d)


def kernel(hidden_states, router_logits, w1, w2):
    from concourse.bass_utils import run_bass_kernel_spmd

    nc, in_maps, meta = prepare(hidden_states, router_logits, w1, w2)
    res = run_bass_kernel_spmd(nc, in_maps, core_ids=list(range(N_CORES)))
    LAST_RUN["capacity"] = meta["C"]
    LAST_RUN["counts"] = meta["counts"]
    return finish(res.results, meta)


# revision 9
# speedup vs baseline: 5.8421x; 5.8421x over previous
"""Top-1 MoE (8 experts) expert-parallel kernel for Trainium2, 8 NeuronCores.

Strategy:
  - Host: argmax(router_logits) -> per-token expert id; gather each expert's
    tokens (the "all-to-all dispatch" happens host-side since we receive full
    inputs and return full outputs).
  - Device (SPMD, one expert per core): dense 2-GEMM SiLU MLP in bf16 with
    fp32 PSUM accumulation. Weights are streamed through SBUF; activations
    (x, h) are SBUF-resident.
  - Host: scatter each expert's outputs back to token order ("combine").

Per-core problem: x[C, D] @ w1[F, D].T -> silu -> @ w2[D, F].T, with
D=2048, F=4096, C = padded max token count per expert (multiple of 128).

Device layouts (partition-major so every DMA is a plain slice):
  xt  [128, 16, C]  bf16   xt[p, ko, t]  = x[t, ko*128+p]        (lhs-T of x)
  w1t [128, 16, F]  bf16   w1t[p, ko, f] = w1[f, ko*128+p]       (k-major w1)
  w2t [128, 32, D]  bf16   w2t[p, ko, d] = w2[d, ko*128+p]       (k-major w2)
  yt  [128, 16, C]  f32    yt[p, do, t]  = y[t, do*128+p]
"""

import numpy as np
import ml_dtypes

BF16 = ml_dtypes.bfloat16

P = 128
D = 2048
F = 4096
E = 8
N_CORES = 8
TCHUNK = 512  # token chunk = matmul free dim (one PSUM bank of fp32)
W1B = 512     # GEMM1 weight block width (columns of F per streamed tile)
W2B = 256     # GEMM2 weight block width (columns of D per streamed tile)

KO1 = D // P  # 16 contraction tiles for GEMM1
KO2 = F // P  # 32 contraction tiles for GEMM2

_BUILD_CACHE = {}


def _token_chunks(C):
    out = []
    t0 = 0
    while t0 < C:
        tw = min(TCHUNK, C - t0)
        out.append((t0, tw))
        t0 += tw
    return out


def build_nc(C, act="silu", reps=1):
    """Build + compile the per-core Bass program for token capacity C.

    reps > 1 repeats the whole compute (for slope-based HW timing); the
    result is identical since the computation is idempotent.
    """
    key = (C, act, reps)
    if key in _BUILD_CACHE:
        return _BUILD_CACHE[key]

    import concourse.bacc as bacc
    import concourse.mybir as mybir
    from concourse import tile

    dt = mybir.dt
    act_fn = {
        "silu": mybir.ActivationFunctionType.Silu,
        "sigmoid": mybir.ActivationFunctionType.Sigmoid,
    }[act]
    nc = bacc.Bacc("TRN2", target_bir_lowering=False, debug=False)

    xt_d = nc.dram_tensor("xt", [P, KO1, C], dt.bfloat16, kind="ExternalInput")
    w1t_d = nc.dram_tensor("w1t", [P, KO1, F], dt.bfloat16, kind="ExternalInput")
    w2t_d = nc.dram_tensor("w2t", [P, KO2, D], dt.bfloat16, kind="ExternalInput")
    yt_d = nc.dram_tensor("yt", [P, KO1, C], dt.float32, kind="ExternalOutput")

    chunks = _token_chunks(C)
    N1 = F // W1B
    N2 = D // W2B

    with tile.TileContext(nc) as tc:
        with (
            tc.tile_pool(name="xpool", bufs=1) as xpool,
            tc.tile_pool(name="hpool", bufs=1) as hpool,
            tc.tile_pool(name="wpool", bufs=2) as wpool,
            tc.tile_pool(name="ypool", bufs=4) as ypool,
            tc.tile_pool(name="cpool", bufs=1) as cpool,
            tc.tile_pool(name="pspool", bufs=4, space="PSUM") as pspool,
        ):
            zbias = cpool.tile([P, 1], dt.float32)
            nc.any.memset(zbias[:], 0.0)

            x_sb = xpool.tile([P, KO1, C], dt.bfloat16)
            h_sb = hpool.tile([P, KO2, C], dt.bfloat16)

            # Load x by token-chunk so GEMM1 can start after the first chunk.
            for (t0, tw) in chunks:
                nc.sync.dma_start(x_sb[:, :, t0 : t0 + tw], xt_d[:, :, t0 : t0 + tw])

            for rep in range(reps):
                # GEMM1 + SiLU: h[f, t] = silu(sum_d w1t[d, f] * x[d, t])
                for mb in range(N1):
                    w1_sb = wpool.tile(
                        [P, KO1, W1B], dt.bfloat16, tag="w", name=f"w1_{rep}_{mb}"
                    )
                    nc.sync.dma_start(
                        w1_sb[:], w1t_d[:, :, mb * W1B : (mb + 1) * W1B]
                    )
                    for (t0, tw) in chunks:
                        for ms in range(W1B // P):
                            ps = pspool.tile(
                                [P, TCHUNK],
                                dt.float32,
                                tag="ps",
                                name=f"ps1_{rep}_{mb}_{t0}_{ms}",
                            )
                            for k in range(KO1):
                                nc.tensor.matmul(
                                    ps[:, :tw],
                                    w1_sb[:, k, ms * P : (ms + 1) * P],
                                    x_sb[:, k, t0 : t0 + tw],
                                    start=(k == 0),
                                    stop=(k == KO1 - 1),
                                )
                            fo = mb * (W1B // P) + ms
                            nc.scalar.activation(
                                h_sb[:, fo, t0 : t0 + tw],
                                ps[:, :tw],
                                act_fn,
                                bias=zbias[:],
                            )

                # GEMM2: y[d, t] = sum_f w2t[f, d] * h[f, t]
                for db in range(N2):
                    w2_sb = wpool.tile(
                        [P, KO2, W2B], dt.bfloat16, tag="w", name=f"w2_{rep}_{db}"
                    )
                    nc.sync.dma_start(
                        w2_sb[:], w2t_d[:, :, db * W2B : (db + 1) * W2B]
                    )
                    for (t0, tw) in chunks:
                        for ds in range(W2B // P):
                            ps = pspool.tile(
                                [P, TCHUNK],
                                dt.float32,
                                tag="ps",
                                name=f"ps2_{rep}_{db}_{t0}_{ds}",
                            )
                            for k in range(KO2):
                                nc.tensor.matmul(
                                    ps[:, :tw],
                                    w2_sb[:, k, ds * P : (ds + 1) * P],
                                    h_sb[:, k, t0 : t0 + tw],
                                    start=(k == 0),
                                    stop=(k == KO2 - 1),
                                )
                            do = db * (W2B // P) + ds
                            y_sb = ypool.tile(
                                [P, TCHUNK],
                                dt.float32,
                                tag="y",
                                name=f"y_{rep}_{db}_{t0}_{ds}",
                            )
                            nc.vector.tensor_copy(y_sb[:, :tw], ps[:, :tw])
                            nc.sync.dma_start(
                                yt_d[:, do, t0 : t0 + tw], y_sb[:, :tw]
                            )

    nc.compile()
    _BUILD_CACHE[key] = nc
    return nc


def _pack_tokens(x_e, C):
    """x_e [n, D] f32 -> xt [128, KO1, C] bf16 (zero padded)."""
    n = x_e.shape[0]
    xb = np.zeros((C, D), dtype=BF16)
    xb[:n] = x_e.astype(BF16)
    return np.ascontiguousarray(xb.reshape(C, KO1, P).transpose(2, 1, 0))


def _pack_w1(w1_e):
    """w1_e [F, D] f32 -> [128, KO1, F] bf16."""
    return np.ascontiguousarray(
        w1_e.astype(BF16).reshape(F, KO1, P).transpose(2, 1, 0)
    )


def _pack_w2(w2_e):
    """w2_e [D, F] f32 -> [128, KO2, D] bf16."""
    return np.ascontiguousarray(
        w2_e.astype(BF16).reshape(D, KO2, P).transpose(2, 1, 0)
    )


LAST_RUN = {}


def prepare(hidden_states, router_logits, w1, w2):
    """Host-side routing + packing. Returns (nc, in_maps, meta)."""
    hidden_states = np.asarray(hidden_states)
    router_logits = np.asarray(router_logits)
    w1 = np.asarray(w1)
    w2 = np.asarray(w2)

    b, s, d = hidden_states.shape
    T = b * s
    x = hidden_states.reshape(T, d).astype(np.float32)
    assign = np.argmax(router_logits.reshape(T, E), axis=-1)

    idx = [np.nonzero(assign == e)[0] for e in range(E)]
    counts = np.array([i.size for i in idx])
    C = max(P, int(-(-counts.max() // P)) * P)  # pad capacity to multiple of 128

    nc = build_nc(C)

    in_maps = []
    for e in range(E):
        in_maps.append(
            {
                "xt": _pack_tokens(x[idx[e]], C),
                "w1t": _pack_w1(w1[e]),
                "w2t": _pack_w2(w2[e]),
            }
        )
    meta = {"b": b, "s": s, "d": d, "T": T, "C": C, "idx": idx, "counts": counts}
    return nc, in_maps, meta


def finish(results, meta):
    """Scatter per-expert outputs back to token order."""
    T, d, C = meta["T"], meta["d"], meta["C"]
    out = np.zeros((T, d), dtype=np.float32)
    for e in range(E):
        yt = np.asarray(results[e]["yt"])  # [128, KO1, C] f32
        y_tok = yt.transpose(2, 1, 0).reshape(C, D)
        out[meta["idx"][e]] = y_tok[: meta["counts"][e]]
    return out.reshape(meta["b"], meta["s"], d)


def kernel(hidden_states, router_logits, w1, w2):
    from concourse.bass_utils import run_bass_kernel_spmd

    nc, in_maps, meta = prepare(hidden_states, router_logits, w1, w2)
    res = run_bass_kernel_spmd(nc, in_maps, core_ids=list(range(N_CORES)))
    LAST_RUN["capacity"] = meta["C"]
    LAST_RUN["counts"] = meta["counts"]
    return finish(res.results, meta)


# revision 18
# speedup vs baseline: 49.5165x; 8.4758x over previous
"""Top-1 MoE (8 experts) expert-parallel kernel for Trainium2, 8 NeuronCores.

Strategy:
  - Host: argmax(router_logits) -> per-token expert id; gather each expert's
    tokens (the "all-to-all dispatch" happens host-side since we receive full
    inputs and return full outputs).
  - Device (SPMD, one expert per core): dense 2-GEMM SiLU MLP in bf16 with
    fp32 PSUM accumulation. Weights are streamed through SBUF; activations
    (x, h) are SBUF-resident.
  - Host: scatter each expert's outputs back to token order ("combine").

Per-core problem: x[C, D] @ w1[F, D].T -> silu -> @ w2[D, F].T, with
D=2048, F=4096, C = padded max token count per expert (multiple of 128).

Device layouts (partition-major so every DMA is a plain slice):
  xt  [128, 16, C]  bf16   xt[p, ko, t]  = x[t, ko*128+p]        (lhs-T of x)
  w1t [128, 16, F]  bf16   w1t[p, ko, f] = w1[f, ko*128+p]       (k-major w1)
  w2t [128, 32, D]  bf16   w2t[p, ko, d] = w2[d, ko*128+p]       (k-major w2)
  yt  [128, 16, C]  f32    yt[p, do, t]  = y[t, do*128+p]
"""

import numpy as np
import ml_dtypes

BF16 = ml_dtypes.bfloat16

P = 128
D = 2048
F = 4096
E = 8
N_CORES = 8
TCHUNK = 512  # token chunk = matmul free dim (one PSUM bank of fp32)
W1B = 512     # GEMM1 weight block width (columns of F per streamed tile)
W2B = 256     # GEMM2 weight block width (columns of D per streamed tile)

KO1 = D // P  # 16 contraction tiles for GEMM1
KO2 = F // P  # 32 contraction tiles for GEMM2

_BUILD_CACHE = {}


def _token_chunks(C):
    out = []
    t0 = 0
    while t0 < C:
        tw = min(TCHUNK, C - t0)
        out.append((t0, tw))
        t0 += tw
    return out


def build_nc(C, act="silu", reps=1, loop_reps=None):
    """Build + compile the per-core Bass program for token capacity C.

    reps > 1 unrolls the whole compute; loop_reps wraps one pass in a
    hardware For_i loop (for slope-based HW timing). Results are identical
    since the computation is idempotent.
    """
    key = (C, act, reps, loop_reps)
    if key in _BUILD_CACHE:
        return _BUILD_CACHE[key]

    import concourse.bacc as bacc
    import concourse.mybir as mybir
    from concourse import tile

    dt = mybir.dt
    act_fn = {
        "silu": mybir.ActivationFunctionType.Silu,
        "sigmoid": mybir.ActivationFunctionType.Sigmoid,
    }[act]
    nc = bacc.Bacc("TRN2", target_bir_lowering=False, debug=False)

    xt_d = nc.dram_tensor("xt", [P, KO1, C], dt.bfloat16, kind="ExternalInput")
    w1t_d = nc.dram_tensor("w1t", [P, KO1, F], dt.bfloat16, kind="ExternalInput")
    w2t_d = nc.dram_tensor("w2t", [P, KO2, D], dt.bfloat16, kind="ExternalInput")
    yt_d = nc.dram_tensor("yt", [P, KO1, C], dt.float32, kind="ExternalOutput")

    chunks = _token_chunks(C)
    N1 = F // W1B
    N2 = D // W2B

    with tile.TileContext(nc) as tc:
        with (
            tc.tile_pool(name="xpool", bufs=1) as xpool,
            tc.tile_pool(name="hpool", bufs=1) as hpool,
            tc.tile_pool(name="wpool", bufs=2) as wpool,
            tc.tile_pool(name="ypool", bufs=4) as ypool,
            tc.tile_pool(name="cpool", bufs=1) as cpool,
            tc.tile_pool(name="pspool", bufs=8, space="PSUM") as pspool,
        ):
            zbias = cpool.tile([P, 1], dt.float32)
            nc.any.memset(zbias[:], 0.0)

            x_sb = xpool.tile([P, KO1, C], dt.bfloat16)
            h_sb = hpool.tile([P, KO2, C], dt.bfloat16)

            # Load x by token-chunk so GEMM1 can start after the first chunk.
            for (t0, tw) in chunks:
                nc.sync.dma_start(x_sb[:, :, t0 : t0 + tw], xt_d[:, :, t0 : t0 + tw])

            def one_pass(rep):
                # GEMM1 + SiLU: h[f, t] = silu(sum_d w1t[d, f] * x[d, t])
                # Weight-stationary inner order: for each (m-subtile, k) the
                # lhsT weight tile is reused across all token chunks, so the
                # PE's LDWEIGHTS stays hidden under long matmul streams.
                for mb in range(N1):
                    w1_sb = wpool.tile(
                        [P, KO1, W1B], dt.bfloat16, tag="w", name=f"w1_{rep}_{mb}"
                    )
                    nc.sync.dma_start(
                        w1_sb[:], w1t_d[:, :, mb * W1B : (mb + 1) * W1B]
                    )
                    for ms in range(W1B // P):
                        pss = [
                            pspool.tile(
                                [P, TCHUNK],
                                dt.float32,
                                tag="ps",
                                name=f"ps1_{rep}_{mb}_{ms}_{ci}",
                            )
                            for ci in range(len(chunks))
                        ]
                        for k in range(KO1):
                            for ci, (t0, tw) in enumerate(chunks):
                                nc.tensor.matmul(
                                    pss[ci][:, :tw],
                                    w1_sb[:, k, ms * P : (ms + 1) * P],
                                    x_sb[:, k, t0 : t0 + tw],
                                    start=(k == 0),
                                    stop=(k == KO1 - 1),
                                )
                        fo = mb * (W1B // P) + ms
                        for ci, (t0, tw) in enumerate(chunks):
                            nc.scalar.activation(
                                h_sb[:, fo, t0 : t0 + tw],
                                pss[ci][:, :tw],
                                act_fn,
                                bias=zbias[:],
                            )

                # GEMM2: y[d, t] = sum_f w2t[f, d] * h[f, t]
                for db in range(N2):
                    w2_sb = wpool.tile(
                        [P, KO2, W2B], dt.bfloat16, tag="w", name=f"w2_{rep}_{db}"
                    )
                    nc.sync.dma_start(
                        w2_sb[:], w2t_d[:, :, db * W2B : (db + 1) * W2B]
                    )
                    for ds in range(W2B // P):
                        pss = [
                            pspool.tile(
                                [P, TCHUNK],
                                dt.float32,
                                tag="ps",
                                name=f"ps2_{rep}_{db}_{ds}_{ci}",
                            )
                            for ci in range(len(chunks))
                        ]
                        for k in range(KO2):
                            for ci, (t0, tw) in enumerate(chunks):
                                nc.tensor.matmul(
                                    pss[ci][:, :tw],
                                    w2_sb[:, k, ds * P : (ds + 1) * P],
                                    h_sb[:, k, t0 : t0 + tw],
                                    start=(k == 0),
                                    stop=(k == KO2 - 1),
                                )
                        do = db * (W2B // P) + ds
                        for ci, (t0, tw) in enumerate(chunks):
                            y_sb = ypool.tile(
                                [P, TCHUNK],
                                dt.float32,
                                tag="y",
                                name=f"y_{rep}_{db}_{ds}_{ci}",
                            )
                            nc.vector.tensor_copy(y_sb[:, :tw], pss[ci][:, :tw])
                            nc.sync.dma_start(
                                yt_d[:, do, t0 : t0 + tw], y_sb[:, :tw]
                            )

            if loop_reps is not None and loop_reps > 1:
                with tc.For_i(0, loop_reps, 1):
                    one_pass(0)
            else:
                for rep in range(reps):
                    one_pass(rep)

    nc.compile()
    _BUILD_CACHE[key] = nc
    return nc


def build_nc2(S1, S2, act="silu", loop_reps=None):
    """Two-segment variant: tokens [0:S1] use weight set 0, [S1:S1+S2] use
    weight set 1 (per-core data). Lets the host balance load by packing up
    to two (expert, token-group) bins per core."""
    key = ("2seg", S1, S2, act, loop_reps)
    if key in _BUILD_CACHE:
        return _BUILD_CACHE[key]

    import concourse.bacc as bacc
    import concourse.mybir as mybir
    from concourse import tile

    C = S1 + S2
    dt = mybir.dt
    act_fn = {
        "silu": mybir.ActivationFunctionType.Silu,
        "sigmoid": mybir.ActivationFunctionType.Sigmoid,
    }[act]
    nc = bacc.Bacc("TRN2", target_bir_lowering=False, debug=False)

    xt_d = nc.dram_tensor("xt", [P, KO1, C], dt.bfloat16, kind="ExternalInput")
    w1t_d = nc.dram_tensor("w1t", [2, P, KO1, F], dt.bfloat16, kind="ExternalInput")
    w2t_d = nc.dram_tensor("w2t", [2, P, KO2, D], dt.bfloat16, kind="ExternalInput")
    yt_d = nc.dram_tensor("yt", [P, KO1, C], dt.float32, kind="ExternalOutput")

    segs = [(0, S1, 0), (S1, S2, 1)]
    seg_chunks = []
    for (base, size, w) in segs:
        t0 = 0
        while t0 < size:
            tw = min(TCHUNK, size - t0)
            seg_chunks.append((w, base + t0, tw))
            t0 += tw

    N1 = F // W1B
    N2 = D // W2B

    with tile.TileContext(nc) as tc:
        with (
            tc.tile_pool(name="xpool", bufs=1) as xpool,
            tc.tile_pool(name="hpool", bufs=1) as hpool,
            tc.tile_pool(name="wpool", bufs=2) as wpool,
            tc.tile_pool(name="ypool", bufs=4) as ypool,
            tc.tile_pool(name="cpool", bufs=1) as cpool,
            tc.tile_pool(name="pspool", bufs=8, space="PSUM") as pspool,
        ):
            zbias = cpool.tile([P, 1], dt.float32)
            nc.any.memset(zbias[:], 0.0)

            x_sb = xpool.tile([P, KO1, C], dt.bfloat16)
            h_sb = hpool.tile([P, KO2, C], dt.bfloat16)

            for (w, t0, tw) in seg_chunks:
                nc.sync.dma_start(x_sb[:, :, t0 : t0 + tw], xt_d[:, :, t0 : t0 + tw])

            def one_pass(rep):
                for seg, (base, size, w) in enumerate(segs):
                    chunks = [(t0, tw) for (ws, t0, tw) in seg_chunks if ws == w]
                    for mb in range(N1):
                        w1_sb = wpool.tile(
                            [P, KO1, W1B],
                            dt.bfloat16,
                            tag="w",
                            name=f"w1_{rep}_{seg}_{mb}",
                        )
                        nc.sync.dma_start(
                            w1_sb[:], w1t_d[w, :, :, mb * W1B : (mb + 1) * W1B]
                        )
                        for ms in range(W1B // P):
                            pss = [
                                pspool.tile(
                                    [P, TCHUNK],
                                    dt.float32,
                                    tag="ps",
                                    name=f"ps1_{rep}_{seg}_{mb}_{ms}_{ci}",
                                )
                                for ci in range(len(chunks))
                            ]
                            for k in range(KO1):
                                for ci, (t0, tw) in enumerate(chunks):
                                    nc.tensor.matmul(
                                        pss[ci][:, :tw],
                                        w1_sb[:, k, ms * P : (ms + 1) * P],
                                        x_sb[:, k, t0 : t0 + tw],
                                        start=(k == 0),
                                        stop=(k == KO1 - 1),
                                    )
                            fo = mb * (W1B // P) + ms
                            for ci, (t0, tw) in enumerate(chunks):
                                nc.scalar.activation(
                                    h_sb[:, fo, t0 : t0 + tw],
                                    pss[ci][:, :tw],
                                    act_fn,
                                    bias=zbias[:],
                                )
                for seg, (base, size, w) in enumerate(segs):
                    chunks = [(t0, tw) for (ws, t0, tw) in seg_chunks if ws == w]
                    for db in range(N2):
                        w2_sb = wpool.tile(
                            [P, KO2, W2B],
                            dt.bfloat16,
                            tag="w",
                            name=f"w2_{rep}_{seg}_{db}",
                        )
                        nc.sync.dma_start(
                            w2_sb[:], w2t_d[w, :, :, db * W2B : (db + 1) * W2B]
                        )
                        for ds in range(W2B // P):
                            pss = [
                                pspool.tile(
                                    [P, TCHUNK],
                                    dt.float32,
                                    tag="ps",
                                    name=f"ps2_{rep}_{seg}_{db}_{ds}_{ci}",
                                )
                                for ci in range(len(chunks))
                            ]
                            for k in range(KO2):
                                for ci, (t0, tw) in enumerate(chunks):
                                    nc.tensor.matmul(
                                        pss[ci][:, :tw],
                                        w2_sb[:, k, ds * P : (ds + 1) * P],
                                        h_sb[:, k, t0 : t0 + tw],
                                        start=(k == 0),
                                        stop=(k == KO2 - 1),
                                    )
                            do = db * (W2B // P) + ds
                            for ci, (t0, tw) in enumerate(chunks):
                                y_sb = ypool.tile(
                                    [P, TCHUNK],
                                    dt.float32,
                                    tag="y",
                                    name=f"y_{rep}_{seg}_{db}_{ds}_{ci}",
                                )
                                nc.vector.tensor_copy(y_sb[:, :tw], pss[ci][:, :tw])
                                nc.sync.dma_start(
                                    yt_d[:, do, t0 : t0 + tw], y_sb[:, :tw]
                                )

            if loop_reps is not None and loop_reps > 1:
                with tc.For_i(0, loop_reps, 1):
                    one_pass(0)
            else:
                one_pass(0)

    nc.compile()
    _BUILD_CACHE[key] = nc
    return nc


def _solve_bins_full(counts, c_min, c_max):
    """Search (S1, S2), S1+S2 minimal, with a feasible single-expert bin
    assignment (8 bins of each size). Returns (S1, S2, alloc) or None."""
    for c_bal in range(c_min, c_max, 128):
        for s2 in range(128, c_bal // 2 + 1, 128):
            s1 = c_bal - s2
            alloc = _solve_bins_levels(counts, s1, s2)
            if alloc is not None:
                return (s1, s2, alloc)
    return None


def _solve_bins_levels(counts, s1, s2):
    """Like _solve_bins but keeps per-level DP tables for backtracking."""
    n = len(counts)
    levels = [{(0, 0): None}]
    for e, c in enumerate(counts):
        opts = []
        for k1 in range(9):
            for k2 in range(9):
                if (
                    k1 * s1 + k2 * s2 >= c
                    and (k1 == 0 or (k1 - 1) * s1 + k2 * s2 < c)
                    and (k2 == 0 or k1 * s1 + (k2 - 1) * s2 < c)
                ):
                    opts.append((k1, k2))
        new = {}
        for (u1, u2), _ in levels[-1].items():
            for (k1, k2) in opts:
                if u1 + k1 <= 8 and u2 + k2 <= 8:
                    ns = (u1 + k1, u2 + k2)
                    if ns not in new:
                        new[ns] = ((u1, u2), (k1, k2))
        if not new:
            return None
        levels.append(new)
    state = next(iter(levels[-1]))
    alloc = [None] * n
    for e in range(n - 1, -1, -1):
        prev, ks = levels[e + 1][state]
        alloc[e] = ks
        state = prev
    return alloc


def _pack_tokens(x_e, C):
    """x_e [n, D] f32 -> xt [128, KO1, C] bf16 (zero padded)."""
    n = x_e.shape[0]
    xb = np.zeros((C, D), dtype=BF16)
    xb[:n] = x_e.astype(BF16)
    return np.ascontiguousarray(xb.reshape(C, KO1, P).transpose(2, 1, 0))


def _pack_w1(w1_e):
    """w1_e [F, D] f32 -> [128, KO1, F] bf16."""
    return np.ascontiguousarray(
        w1_e.astype(BF16).reshape(F, KO1, P).transpose(2, 1, 0)
    )


def _pack_w2(w2_e):
    """w2_e [D, F] f32 -> [128, KO2, D] bf16."""
    return np.ascontiguousarray(
        w2_e.astype(BF16).reshape(D, KO2, P).transpose(2, 1, 0)
    )


LAST_RUN = {}


def prepare(hidden_states, router_logits, w1, w2):
    """Host-side routing + packing. Returns (nc, in_maps, meta)."""
    hidden_states = np.asarray(hidden_states)
    router_logits = np.asarray(router_logits)
    w1 = np.asarray(w1)
    w2 = np.asarray(w2)

    b, s, d = hidden_states.shape
    T = b * s
    x = hidden_states.reshape(T, d).astype(np.float32)
    assign = np.argmax(router_logits.reshape(T, E), axis=-1)

    idx = [np.nonzero(assign == e)[0] for e in range(E)]
    counts = [int(i.size) for i in idx]
    single_C = max(P, int(-(-max(counts) // P)) * P)

    c_min = max(2 * P, int(-(-T // (N_CORES * P))) * P)
    sol = _solve_bins_full(counts, c_min, single_C)

    w1_packed = {}
    w2_packed = {}

    def packed(e):
        if e not in w1_packed:
            w1_packed[e] = _pack_w1(w1[e])
            w2_packed[e] = _pack_w2(w2[e])
        return w1_packed[e], w2_packed[e]

    if sol is None:
        # One expert per core, capacity = padded max count.
        C = single_C
        nc = build_nc(C)
        in_maps = []
        for e in range(E):
            p1, p2 = packed(e)
            in_maps.append({"xt": _pack_tokens(x[idx[e]], C), "w1t": p1, "w2t": p2})
        meta = {
            "mode": "1seg", "b": b, "s": s, "d": d, "T": T, "C": C,
            "idx": idx, "counts": counts,
        }
        return nc, in_maps, meta

    # Balanced 2-segment packing.
    S1, S2, alloc = sol
    C = S1 + S2
    nc = build_nc2(S1, S2)

    # Build bins: each expert's tokens split across its bins (S1 bins first).
    bins1, bins2 = [], []
    for e in range(E):
        k1, k2 = alloc[e]
        pos = 0
        for _ in range(k1):
            take = min(S1, counts[e] - pos)
            bins1.append((e, idx[e][pos : pos + take]))
            pos += take
        for _ in range(k2):
            take = min(S2, counts[e] - pos)
            bins2.append((e, idx[e][pos : pos + take]))
            pos += take
        assert pos == counts[e]
    while len(bins1) < N_CORES:
        bins1.append((0, np.zeros(0, dtype=np.int64)))
    while len(bins2) < N_CORES:
        bins2.append((0, np.zeros(0, dtype=np.int64)))

    in_maps = []
    core_bins = []
    for c in range(N_CORES):
        (eA, idxA), (eB, idxB) = bins1[c], bins2[c]
        xb = np.zeros((C, D), dtype=BF16)
        xb[: len(idxA)] = x[idxA].astype(BF16)
        xb[S1 : S1 + len(idxB)] = x[idxB].astype(BF16)
        xt = np.ascontiguousarray(xb.reshape(C, KO1, P).transpose(2, 1, 0))
        p1A, p2A = packed(eA)
        p1B, p2B = packed(eB)
        in_maps.append(
            {
                "xt": xt,
                "w1t": np.ascontiguousarray(np.stack([p1A, p1B])),
                "w2t": np.ascontiguousarray(np.stack([p2A, p2B])),
            }
        )
        core_bins.append((idxA, idxB))

    meta = {
        "mode": "2seg", "b": b, "s": s, "d": d, "T": T, "C": C,
        "S1": S1, "S2": S2, "core_bins": core_bins,
        "idx": idx, "counts": counts,
    }
    return nc, in_maps, meta


def finish(results, meta):
    """Scatter per-core outputs back to token order."""
    T, d, C = meta["T"], meta["d"], meta["C"]
    out = np.zeros((T, d), dtype=np.float32)
    if meta["mode"] == "1seg":
        for e in range(E):
            yt = np.asarray(results[e]["yt"])  # [128, KO1, C] f32
            y_tok = yt.transpose(2, 1, 0).reshape(C, D)
            out[meta["idx"][e]] = y_tok[: meta["counts"][e]]
    else:
        S1 = meta["S1"]
        for c in range(N_CORES):
            idxA, idxB = meta["core_bins"][c]
            yt = np.asarray(results[c]["yt"])
            y_tok = yt.transpose(2, 1, 0).reshape(C, D)
            out[idxA] = y_tok[: len(idxA)]
            out[idxB] = y_tok[S1 : S1 + len(idxB)]
    return out.reshape(meta["b"], meta["s"], d)


def kernel(hidden_states, router_logits, w1, w2):
    from concourse.bass_utils import run_bass_kernel_spmd

    nc, in_maps, meta = prepare(hidden_states, router_logits, w1, w2)
    res = run_bass_kernel_spmd(nc, in_maps, core_ids=list(range(N_CORES)))
    LAST_RUN["capacity"] = meta["C"]
    LAST_RUN["counts"] = meta["counts"]
    return finish(res.results, meta)


# revision 22
# speedup vs baseline: 52.2492x; 1.0552x over previous
"""Top-1 MoE (8 experts) expert-parallel kernel for Trainium2, 8 NeuronCores.

Strategy:
  - Host: argmax(router_logits) -> per-token expert id; gather each expert's
    tokens (the "all-to-all dispatch" happens host-side since we receive full
    inputs and return full outputs).
  - Device (SPMD, one expert per core): dense 2-GEMM SiLU MLP in bf16 with
    fp32 PSUM accumulation. Weights are streamed through SBUF; activations
    (x, h) are SBUF-resident.
  - Host: scatter each expert's outputs back to token order ("combine").

Per-core problem: x[C, D] @ w1[F, D].T -> silu -> @ w2[D, F].T, with
D=2048, F=4096, C = padded max token count per expert (multiple of 128).

Device layouts (partition-major so every DMA is a plain slice):
  xt  [128, 16, C]  bf16   xt[p, ko, t]  = x[t, ko*128+p]        (lhs-T of x)
  w1t [128, 16, F]  bf16   w1t[p, ko, f] = w1[f, ko*128+p]       (k-major w1)
  w2t [128, 32, D]  bf16   w2t[p, ko, d] = w2[d, ko*128+p]       (k-major w2)
  yt  [128, 16, C]  f32    yt[p, do, t]  = y[t, do*128+p]
"""

import numpy as np
import ml_dtypes

BF16 = ml_dtypes.bfloat16

P = 128
D = 2048
F = 4096
E = 8
N_CORES = 8
TCHUNK = 512  # token chunk = matmul free dim (one PSUM bank of fp32)
W1B = 512     # GEMM1 weight block width (columns of F per streamed tile)
W2B = 256     # GEMM2 weight block width (columns of D per streamed tile)

KO1 = D // P  # 16 contraction tiles for GEMM1
KO2 = F // P  # 32 contraction tiles for GEMM2

_BUILD_CACHE = {}


def _token_chunks(C):
    out = []
    t0 = 0
    while t0 < C:
        tw = min(TCHUNK, C - t0)
        out.append((t0, tw))
        t0 += tw
    return out


def build_nc(C, act="silu", reps=1, loop_reps=None):
    """Build + compile the per-core Bass program for token capacity C.

    reps > 1 unrolls the whole compute; loop_reps wraps one pass in a
    hardware For_i loop (for slope-based HW timing). Results are identical
    since the computation is idempotent.
    """
    key = (C, act, reps, loop_reps)
    if key in _BUILD_CACHE:
        return _BUILD_CACHE[key]

    import concourse.bacc as bacc
    import concourse.mybir as mybir
    from concourse import tile

    dt = mybir.dt
    act_fn = {
        "silu": mybir.ActivationFunctionType.Silu,
        "sigmoid": mybir.ActivationFunctionType.Sigmoid,
    }[act]
    nc = bacc.Bacc("TRN2", target_bir_lowering=False, debug=False)

    xt_d = nc.dram_tensor("xt", [P, KO1, C], dt.bfloat16, kind="ExternalInput")
    w1t_d = nc.dram_tensor("w1t", [P, KO1, F], dt.bfloat16, kind="ExternalInput")
    w2t_d = nc.dram_tensor("w2t", [P, KO2, D], dt.bfloat16, kind="ExternalInput")
    yt_d = nc.dram_tensor("yt", [P, KO1, C], dt.float32, kind="ExternalOutput")

    chunks = _token_chunks(C)
    N1 = F // W1B
    N2 = D // W2B

    with tile.TileContext(nc) as tc:
        with (
            tc.tile_pool(name="xpool", bufs=1) as xpool,
            tc.tile_pool(name="hpool", bufs=1) as hpool,
            tc.tile_pool(name="wpool", bufs=3) as wpool,
            tc.tile_pool(name="ypool", bufs=4) as ypool,
            tc.tile_pool(name="cpool", bufs=1) as cpool,
            tc.tile_pool(name="pspool", bufs=8, space="PSUM") as pspool,
        ):
            zbias = cpool.tile([P, 1], dt.float32)
            nc.any.memset(zbias[:], 0.0)

            x_sb = xpool.tile([P, KO1, C], dt.bfloat16)
            h_sb = hpool.tile([P, KO2, C], dt.bfloat16)

            # Load x by token-chunk so GEMM1 can start after the first chunk.
            for (t0, tw) in chunks:
                nc.sync.dma_start(x_sb[:, :, t0 : t0 + tw], xt_d[:, :, t0 : t0 + tw])

            def one_pass(rep):
                # GEMM1 + SiLU: h[f, t] = silu(sum_d w1t[d, f] * x[d, t])
                for mb in range(N1):
                    w1_sb = wpool.tile(
                        [P, KO1, W1B], dt.bfloat16, tag="w", name=f"w1_{rep}_{mb}"
                    )
                    nc.sync.dma_start(
                        w1_sb[:], w1t_d[:, :, mb * W1B : (mb + 1) * W1B]
                    )
                    for (t0, tw) in chunks:
                        for ms in range(W1B // P):
                            ps = pspool.tile(
                                [P, TCHUNK],
                                dt.float32,
                                tag="ps",
                                name=f"ps1_{rep}_{mb}_{t0}_{ms}",
                            )
                            for k in range(KO1):
                                nc.tensor.matmul(
                                    ps[:, :tw],
                                    w1_sb[:, k, ms * P : (ms + 1) * P],
                                    x_sb[:, k, t0 : t0 + tw],
                                    start=(k == 0),
                                    stop=(k == KO1 - 1),
                                )
                            fo = mb * (W1B // P) + ms
                            nc.scalar.activation(
                                h_sb[:, fo, t0 : t0 + tw],
                                ps[:, :tw],
                                act_fn,
                                bias=zbias[:],
                            )

                # GEMM2: y[d, t] = sum_f w2t[f, d] * h[f, t]
                for db in range(N2):
                    w2_sb = wpool.tile(
                        [P, KO2, W2B], dt.bfloat16, tag="w", name=f"w2_{rep}_{db}"
                    )
                    nc.sync.dma_start(
                        w2_sb[:], w2t_d[:, :, db * W2B : (db + 1) * W2B]
                    )
                    for (t0, tw) in chunks:
                        for ds in range(W2B // P):
                            ps = pspool.tile(
                                [P, TCHUNK],
                                dt.float32,
                                tag="ps",
                                name=f"ps2_{rep}_{db}_{t0}_{ds}",
                            )
                            for k in range(KO2):
                                nc.tensor.matmul(
                                    ps[:, :tw],
                                    w2_sb[:, k, ds * P : (ds + 1) * P],
                                    h_sb[:, k, t0 : t0 + tw],
                                    start=(k == 0),
                                    stop=(k == KO2 - 1),
                                )
                            do = db * (W2B // P) + ds
                            y_sb = ypool.tile(
                                [P, TCHUNK],
                                dt.float32,
                                tag="y",
                                name=f"y_{rep}_{db}_{t0}_{ds}",
                            )
                            nc.vector.tensor_copy(y_sb[:, :tw], ps[:, :tw])
                            nc.sync.dma_start(
                                yt_d[:, do, t0 : t0 + tw], y_sb[:, :tw]
                            )

            if loop_reps is not None and loop_reps > 1:
                with tc.For_i(0, loop_reps, 1):
                    one_pass(0)
            else:
                for rep in range(reps):
                    one_pass(rep)

    nc.compile()
    _BUILD_CACHE[key] = nc
    return nc


def build_nc2(S1, S2, act="silu", loop_reps=None):
    """Two-segment variant: tokens [0:S1] use weight set 0, [S1:S1+S2] use
    weight set 1 (per-core data). Lets the host balance load by packing up
    to two (expert, token-group) bins per core."""
    key = ("2seg", S1, S2, act, loop_reps)
    if key in _BUILD_CACHE:
        return _BUILD_CACHE[key]

    import concourse.bacc as bacc
    import concourse.mybir as mybir
    from concourse import tile

    C = S1 + S2
    dt = mybir.dt
    act_fn = {
        "silu": mybir.ActivationFunctionType.Silu,
        "sigmoid": mybir.ActivationFunctionType.Sigmoid,
    }[act]
    nc = bacc.Bacc("TRN2", target_bir_lowering=False, debug=False)

    xt_d = nc.dram_tensor("xt", [P, KO1, C], dt.bfloat16, kind="ExternalInput")
    w1t_d = nc.dram_tensor("w1t", [2, P, KO1, F], dt.bfloat16, kind="ExternalInput")
    w2t_d = nc.dram_tensor("w2t", [2, P, KO2, D], dt.bfloat16, kind="ExternalInput")
    yt_d = nc.dram_tensor("yt", [P, KO1, C], dt.float32, kind="ExternalOutput")

    segs = [(0, S1, 0), (S1, S2, 1)]
    seg_chunks = []
    for (base, size, w) in segs:
        t0 = 0
        while t0 < size:
            tw = min(TCHUNK, size - t0)
            seg_chunks.append((w, base + t0, tw))
            t0 += tw

    N1 = F // W1B
    N2 = D // W2B

    with tile.TileContext(nc) as tc:
        with (
            tc.tile_pool(name="xpool", bufs=1) as xpool,
            tc.tile_pool(name="hpool", bufs=1) as hpool,
            tc.tile_pool(name="wpool", bufs=2) as wpool,
            tc.tile_pool(name="ypool", bufs=4) as ypool,
            tc.tile_pool(name="cpool", bufs=1) as cpool,
            tc.tile_pool(name="pspool", bufs=8, space="PSUM") as pspool,
        ):
            zbias = cpool.tile([P, 1], dt.float32)
            nc.any.memset(zbias[:], 0.0)

            x_sb = xpool.tile([P, KO1, C], dt.bfloat16)
            h_sb = hpool.tile([P, KO2, C], dt.bfloat16)

            for (w, t0, tw) in seg_chunks:
                nc.sync.dma_start(x_sb[:, :, t0 : t0 + tw], xt_d[:, :, t0 : t0 + tw])

            def one_pass(rep):
                for seg, (base, size, w) in enumerate(segs):
                    chunks = [(t0, tw) for (ws, t0, tw) in seg_chunks if ws == w]
                    for mb in range(N1):
                        w1_sb = wpool.tile(
                            [P, KO1, W1B],
                            dt.bfloat16,
                            tag="w",
                            name=f"w1_{rep}_{seg}_{mb}",
                        )
                        nc.sync.dma_start(
                            w1_sb[:], w1t_d[w, :, :, mb * W1B : (mb + 1) * W1B]
                        )
                        for ms in range(W1B // P):
                            pss = [
                                pspool.tile(
                                    [P, TCHUNK],
                                    dt.float32,
                                    tag="ps",
                                    name=f"ps1_{rep}_{seg}_{mb}_{ms}_{ci}",
                                )
                                for ci in range(len(chunks))
                            ]
                            for k in range(KO1):
                                for ci, (t0, tw) in enumerate(chunks):
                                    nc.tensor.matmul(
                                        pss[ci][:, :tw],
                                        w1_sb[:, k, ms * P : (ms + 1) * P],
                                        x_sb[:, k, t0 : t0 + tw],
                                        start=(k == 0),
                                        stop=(k == KO1 - 1),
                                    )
                            fo = mb * (W1B // P) + ms
                            for ci, (t0, tw) in enumerate(chunks):
                                nc.scalar.activation(
                                    h_sb[:, fo, t0 : t0 + tw],
                                    pss[ci][:, :tw],
                                    act_fn,
                                    bias=zbias[:],
                                )
                for seg, (base, size, w) in enumerate(segs):
                    chunks = [(t0, tw) for (ws, t0, tw) in seg_chunks if ws == w]
                    for db in range(N2):
                        w2_sb = wpool.tile(
                            [P, KO2, W2B],
                            dt.bfloat16,
                            tag="w",
                            name=f"w2_{rep}_{seg}_{db}",
                        )
                        nc.sync.dma_start(
                            w2_sb[:], w2t_d[w, :, :, db * W2B : (db + 1) * W2B]
                        )
                        for ds in range(W2B // P):
                            pss = [
                                pspool.tile(
                                    [P, TCHUNK],
                                    dt.float32,
                                    tag="ps",
                                    name=f"ps2_{rep}_{seg}_{db}_{ds}_{ci}",
                                )
                                for ci in range(len(chunks))
                            ]
                            for k in range(KO2):
                                for ci, (t0, tw) in enumerate(chunks):
                                    nc.tensor.matmul(
                                        pss[ci][:, :tw],
                                        w2_sb[:, k, ds * P : (ds + 1) * P],
                                        h_sb[:, k, t0 : t0 + tw],
                                        start=(k == 0),
                                        stop=(k == KO2 - 1),
                                    )
                            do = db * (W2B // P) + ds
                            for ci, (t0, tw) in enumerate(chunks):
                                y_sb = ypool.tile(
                                    [P, TCHUNK],
                                    dt.float32,
                                    tag="y",
                                    name=f"y_{rep}_{seg}_{db}_{ds}_{ci}",
                                )
                                nc.vector.tensor_copy(y_sb[:, :tw], pss[ci][:, :tw])
                                nc.sync.dma_start(
                                    yt_d[:, do, t0 : t0 + tw], y_sb[:, :tw]
                                )

            if loop_reps is not None and loop_reps > 1:
                with tc.For_i(0, loop_reps, 1):
                    one_pass(0)
            else:
                one_pass(0)

    nc.compile()
    _BUILD_CACHE[key] = nc
    return nc


def _solve_bins_full(counts, c_min, c_max):
    """Search (S1, S2), S1+S2 minimal, with a feasible single-expert bin
    assignment (8 bins of each size). Returns (S1, S2, alloc) or None."""
    for c_bal in range(c_min, c_max, 128):
        for s2 in range(128, c_bal // 2 + 1, 128):
            s1 = c_bal - s2
            alloc = _solve_bins_levels(counts, s1, s2)
            if alloc is not None:
                return (s1, s2, alloc)
    return None


def _solve_bins_levels(counts, s1, s2):
    """Like _solve_bins but keeps per-level DP tables for backtracking."""
    n = len(counts)
    levels = [{(0, 0): None}]
    for e, c in enumerate(counts):
        opts = []
        for k1 in range(9):
            for k2 in range(9):
                if (
                    k1 * s1 + k2 * s2 >= c
                    and (k1 == 0 or (k1 - 1) * s1 + k2 * s2 < c)
                    and (k2 == 0 or k1 * s1 + (k2 - 1) * s2 < c)
                ):
                    opts.append((k1, k2))
        new = {}
        for (u1, u2), _ in levels[-1].items():
            for (k1, k2) in opts:
                if u1 + k1 <= 8 and u2 + k2 <= 8:
                    ns = (u1 + k1, u2 + k2)
                    if ns not in new:
                        new[ns] = ((u1, u2), (k1, k2))
        if not new:
            return None
        levels.append(new)
    state = next(iter(levels[-1]))
    alloc = [None] * n
    for e in range(n - 1, -1, -1):
        prev, ks = levels[e + 1][state]
        alloc[e] = ks
        state = prev
    return alloc


def _pack_tokens(x_e, C):
    """x_e [n, D] f32 -> xt [128, KO1, C] bf16 (zero padded)."""
    n = x_e.shape[0]
    xb = np.zeros((C, D), dtype=BF16)
    xb[:n] = x_e.astype(BF16)
    return np.ascontiguousarray(xb.reshape(C, KO1, P).transpose(2, 1, 0))


def _pack_w1(w1_e):
    """w1_e [F, D] f32 -> [128, KO1, F] bf16."""
    return np.ascontiguousarray(
        w1_e.astype(BF16).reshape(F, KO1, P).transpose(2, 1, 0)
    )


def _pack_w2(w2_e):
    """w2_e [D, F] f32 -> [128, KO2, D] bf16."""
    return np.ascontiguousarray(
        w2_e.astype(BF16).reshape(D, KO2, P).transpose(2, 1, 0)
    )


LAST_RUN = {}


def prepare(hidden_states, router_logits, w1, w2):
    """Host-side routing + packing. Returns (nc, in_maps, meta)."""
    hidden_states = np.asarray(hidden_states)
    router_logits = np.asarray(router_logits)
    w1 = np.asarray(w1)
    w2 = np.asarray(w2)

    b, s, d = hidden_states.shape
    T = b * s
    x = hidden_states.reshape(T, d).astype(np.float32)
    assign = np.argmax(router_logits.reshape(T, E), axis=-1)

    idx = [np.nonzero(assign == e)[0] for e in range(E)]
    counts = [int(i.size) for i in idx]
    # Capacity is a matmul free-dim, so it needn't be a multiple of 128 —
    # exact max count avoids computing padded tokens.
    single_C = max(P, max(counts))

    # 2-segment packing pays a fixed overhead (short-N tail chunks, a second
    # weight stream); measured on HW it only wins when it saves >=2 tiles of
    # per-core capacity.
    c_min = max(2 * P, int(-(-T // (N_CORES * P))) * P)
    sol = _solve_bins_full(counts, c_min, single_C - P)

    w1_packed = {}
    w2_packed = {}

    def packed(e):
        if e not in w1_packed:
            w1_packed[e] = _pack_w1(w1[e])
            w2_packed[e] = _pack_w2(w2[e])
        return w1_packed[e], w2_packed[e]

    if sol is None:
        # One expert per core, capacity = padded max count.
        C = single_C
        nc = build_nc(C)
        in_maps = []
        for e in range(E):
            p1, p2 = packed(e)
            in_maps.append({"xt": _pack_tokens(x[idx[e]], C), "w1t": p1, "w2t": p2})
        meta = {
            "mode": "1seg", "b": b, "s": s, "d": d, "T": T, "C": C,
            "idx": idx, "counts": counts,
        }
        return nc, in_maps, meta

    # Balanced 2-segment packing.
    S1, S2, alloc = sol
    C = S1 + S2
    nc = build_nc2(S1, S2)

    # Build bins: each expert's tokens split across its bins (S1 bins first).
    bins1, bins2 = [], []
    for e in range(E):
        k1, k2 = alloc[e]
        pos = 0
        for _ in range(k1):
            take = min(S1, counts[e] - pos)
            bins1.append((e, idx[e][pos : pos + take]))
            pos += take
        for _ in range(k2):
            take = min(S2, counts[e] - pos)
            bins2.append((e, idx[e][pos : pos + take]))
            pos += take
        assert pos == counts[e]
    while len(bins1) < N_CORES:
        bins1.append((0, np.zeros(0, dtype=np.int64)))
    while len(bins2) < N_CORES:
        bins2.append((0, np.zeros(0, dtype=np.int64)))

    in_maps = []
    core_bins = []
    for c in range(N_CORES):
        (eA, idxA), (eB, idxB) = bins1[c], bins2[c]
        xb = np.zeros((C, D), dtype=BF16)
        xb[: len(idxA)] = x[idxA].astype(BF16)
        xb[S1 : S1 + len(idxB)] = x[idxB].astype(BF16)
        xt = np.ascontiguousarray(xb.reshape(C, KO1, P).transpose(2, 1, 0))
        p1A, p2A = packed(eA)
        p1B, p2B = packed(eB)
        in_maps.append(
            {
                "xt": xt,
                "w1t": np.ascontiguousarray(np.stack([p1A, p1B])),
                "w2t": np.ascontiguousarray(np.stack([p2A, p2B])),
            }
        )
        core_bins.append((idxA, idxB))

    meta = {
        "mode": "2seg", "b": b, "s": s, "d": d, "T": T, "C": C,
        "S1": S1, "S2": S2, "core_bins": core_bins,
        "idx": idx, "counts": counts,
    }
    return nc, in_maps, meta


def finish(results, meta):
    """Scatter per-core outputs back to token order."""
    T, d, C = meta["T"], meta["d"], meta["C"]
    out = np.zeros((T, d), dtype=np.float32)
    if meta["mode"] == "1seg":
        for e in range(E):
            yt = np.asarray(results[e]["yt"])  # [128, KO1, C] f32
            y_tok = yt.transpose(2, 1, 0).reshape(C, D)
            out[meta["idx"][e]] = y_tok[: meta["counts"][e]]
    else:
        S1 = meta["S1"]
        for c in range(N_CORES):
            idxA, idxB = meta["core_bins"][c]
            yt = np.asarray(results[c]["yt"])
            y_tok = yt.transpose(2, 1, 0).reshape(C, D)
            out[idxA] = y_tok[: len(idxA)]
            out[idxB] = y_tok[S1 : S1 + len(idxB)]
    return out.reshape(meta["b"], meta["s"], d)


def kernel(hidden_states, router_logits, w1, w2):
    from concourse.bass_utils import run_bass_kernel_spmd

    nc, in_maps, meta = prepare(hidden_states, router_logits, w1, w2)
    res = run_bass_kernel_spmd(nc, in_maps, core_ids=list(range(N_CORES)))
    LAST_RUN["capacity"] = meta["C"]
    LAST_RUN["counts"] = meta["counts"]
    return finish(res.results, meta)
